# revision 10
# baseline (speedup 1.0000x reference)
"""Trainium2 Bass kernel for the VQ commitment-loss problem.

Math
----
reference loss = 0.25 * mean((codebook[argmin_k dist] - flat)**2)
               = 0.25/(B*T*D) * sum_n min_k ||flat_n - e_k||^2
since the gathered quantized row realizes exactly the min squared distance.

min_k ||f - e||^2 = ||f||^2 + min_k (||e_k||^2 - 2 f.e_k)

Per core (2 of 16 batches):
  - sum_n ||f_n||^2 via the window-count trick:
        sum over tau of cnt(tau) * x_pad[tau]^2   (cnt = #windows containing tau)
  - the min term via bf16 TensorE matmuls: window tiles are the stationary
    operand (K=128 contraction chunks -> fast weight load), the codebook
    scaled by -2 is the moving operand (512 codes per matmul), ||e_k||^2 is
    folded in as one extra contraction row paired with ones in the window
    operand.  Two 128-window subtiles share a 4-bank PSUM tile; one VectorE
    3D min-reduce [128,2,1024] -> [128,2] drains it.

All window data is expanded up-front into eight resident [128,4096] SBUF
tiles (one per contraction-chunk x batch) by 8 large DMAs spread over both
HWDGE rings, so the main loop runs with zero DMA traffic.

Host side only pads/casts/shards inputs and sums the 8 per-core partials.
"""

import numpy as np
import ml_dtypes

B, P, T = 16, 12, 4096
WIN = 41
PAD = (WIN - 1) // 2          # 20
K = 1024
D = P * WIN                   # 492
COMMITMENT_COST = 0.25

NCORES = 8
BC = B // NCORES              # batches per core = 2
TP = T + 2 * PAD              # padded time = 4136
NCHUNK = 4                    # contraction chunks: 3 pellets * 41 taps = 123 rows
CHROWS = 3 * WIN              # 123
NSUB = BC * T // 128          # 64 subtiles of 128 windows per core
NPAIR = NSUB // 2             # 32 PSUM pair-tiles
TCHUNK = TP // 4              # 1034 (xsq layout)
NWARM = 9                     # HAM warmup matmuls

SCALE = COMMITMENT_COST / (B * T * D)

_CACHED = {}


def _build_nc():
    import concourse.bacc as bacc
    import concourse.bass as bass
    import concourse.mybir as mybir
    import concourse.tile as tile

    BF = mybir.dt.bfloat16
    F32 = mybir.dt.float32
    AX = mybir.AxisListType
    OP = mybir.AluOpType

    nc = bacc.Bacc("TRN2", target_bir_lowering=False, debug=False)

    xw_d = nc.dram_tensor("xw", [BC, P, TP], BF, kind="ExternalInput")
    cb_d = nc.dram_tensor("cb", [128, NCHUNK * K], BF, kind="ExternalInput")
    cnt_d = nc.dram_tensor("cnt", [96, TCHUNK], F32, kind="ExternalInput")
    out_d = nc.dram_tensor("out", [1, 1], F32, kind="ExternalOutput")

    with tile.TileContext(nc) as tc:
        with (
            tc.tile_pool(name="cbpool", bufs=1) as cbpool,
            tc.tile_pool(name="wpool", bufs=1) as wpool,
            tc.tile_pool(name="misc", bufs=1) as misc,
        ):
            # ---- HAM warmup: PE busy from t~0 so the clock is 2.4 GHz when
            # the real matmuls start.
            warm_src = misc.tile([128, 512], BF)
            nc.vector.memset(warm_src[:], 0.5)
            with tc.tile_pool(name="pwarm", bufs=1, space="PSUM") as pwarm:
                wps = pwarm.tile([128, 512], F32)
                for _ in range(NWARM):
                    nc.tensor.matmul(
                        wps[:], warm_src[:, 0:128], warm_src[:], start=True, stop=True
                    )

            # ---- resident codebook tile (rows = contraction dims, free = 4K codes)
            cbt = cbpool.tile([128, NCHUNK * K], BF)
            nc.gpsimd.dma_start(cbt[:], cb_d[:])

            ones_bf = misc.tile([128, 1], BF)
            nc.vector.memset(ones_bf[:], 1.0)
            ones_f = misc.tile([128, 1], F32)
            nc.vector.memset(ones_f[:], 1.0)
            ones5 = misc.tile([5, T], BF)
            nc.vector.memset(ones5[:], 1.0)
            mins_buf = misc.tile([128, NSUB], F32)

            # ---- resident full-batch window tiles: wt[c][b][r=(p',w), t] =
            # xw[b, 3c+p', t+w].  Rows 123..127 are ones (row 123 pairs with
            # the codebook-norm row of chunk 0; the rest pair with zeros).
            # One big DMA each, alternating between the two HWDGE rings.
            wt = [
                [
                    wpool.tile([128, T], BF, tag=f"w{c}_{b}", name=f"wt{c}_{b}")
                    for b in range(BC)
                ]
                for c in range(NCHUNK)
            ]
            for b in range(BC):
                for c in range(NCHUNK):
                    eng = nc.sync if (c % 2 == 0) else nc.scalar
                    eng.dma_start(
                        wt[c][b][0:CHROWS, :],
                        bass.AP(
                            xw_d,
                            (b * P + 3 * c) * TP,
                            [[TP, 3], [1, WIN], [1, T]],
                        ),
                    )
                    nc.gpsimd.dma_start(wt[c][b][CHROWS:128, :], ones5[:])

            # ---- prologue: c_k = ||e_k||^2 (bf16) into cbt row 123 of chunk 0
            with (
                tc.tile_pool(name="pre", bufs=1) as pre,
                tc.tile_pool(name="ppre", bufs=1, space="PSUM") as ppre,
            ):
                sq = pre.tile([128, NCHUNK * K], BF)
                nc.scalar.square(sq[:], cbt[:])  # (-2e)^2 = 4 e^2
                chi = pre.tile([1, K], BF)
                for h in range(2):
                    pc = ppre.tile([1, 512], F32, tag=f"pc{h}", name=f"pc{h}")
                    for c in range(NCHUNK):
                        nc.tensor.matmul(
                            pc[:],
                            ones_bf[:],
                            sq[:, c * K + 512 * h : c * K + 512 * (h + 1)],
                            start=(c == 0),
                            stop=(c == NCHUNK - 1),
                        )
                    nc.vector.tensor_scalar_mul(
                        chi[:, 512 * h : 512 * (h + 1)], pc[:], 0.25
                    )
                nc.gpsimd.dma_start(cbt[CHROWS : CHROWS + 1, 0:K], chi[:])

            # ---- sum_n ||f_n||^2 term: sum cnt(tau) * x^2
            xsq_in = misc.tile([96, TCHUNK], BF)
            nc.gpsimd.dma_start(
                xsq_in[:],
                bass.AP(
                    xw_d,
                    0,
                    [[P * TP, BC], [TP, P], [TCHUNK, 4], [1, TCHUNK]],
                ),
            )
            cnt_sb = misc.tile([96, TCHUNK], F32)
            nc.gpsimd.dma_start(cnt_sb[:], cnt_d[:])
            sqx = misc.tile([96, TCHUNK], F32)
            nc.vector.tensor_mul(sqx[:], xsq_in[:], xsq_in[:])
            wsq = misc.tile([96, TCHUNK], F32)
            nc.vector.tensor_mul(wsq[:], sqx[:], cnt_sb[:])
            selfsum = misc.tile([96, 1], F32)
            nc.vector.tensor_reduce(selfsum[:], wsq[:], axis=AX.X, op=OP.add)

            # ---- main loop: 32 pairs of 128-window subtiles
            with tc.tile_pool(name="pmain", bufs=2, space="PSUM") as pmain:
                for pair in range(NPAIR):
                    ps = pmain.tile([128, 2, K], F32, tag="ps", name=f"ps_{pair}")
                    for s in range(2):
                        i = pair * 2 + s            # subtile index
                        b = i // (NSUB // BC)
                        toff = (i % (NSUB // BC)) * 128
                        for h in range(2):
                            for c in range(NCHUNK):
                                nc.tensor.matmul(
                                    ps[:, s, 512 * h : 512 * (h + 1)],
                                    wt[c][b][:, toff : toff + 128],
                                    cbt[:, c * K + 512 * h : c * K + 512 * (h + 1)],
                                    start=(c == 0),
                                    stop=(c == NCHUNK - 1),
                                )
                    nc.vector.tensor_reduce(
                        mins_buf[:, 2 * pair : 2 * pair + 2],
                        ps[:],
                        axis=AX.X,
                        op=OP.min,
                    )

            # ---- finale: grand sum -> scale -> out
            macc = misc.tile([128, 1], F32)
            nc.vector.tensor_reduce(macc[:], mins_buf[:], axis=AX.X, op=OP.add)
            with tc.tile_pool(name="pfin", bufs=1, space="PSUM") as pfin:
                fin = pfin.tile([1, 1], F32)
                nc.tensor.matmul(fin[:], macc[:], ones_f[:], start=True, stop=False)
                nc.tensor.matmul(
                    fin[:], selfsum[:], ones_f[0:96, :], start=False, stop=True
                )
                res = misc.tile([1, 1], F32)
                nc.vector.tensor_scalar_mul(res[:], fin[:], float(SCALE))
                nc.gpsimd.dma_start(out_d[:], res[:])

    nc.compile()
    return nc


def get_nc():
    if "nc" not in _CACHED:
        _CACHED["nc"] = _build_nc()
    return _CACHED["nc"]


def _host_prep(x, codebook):
    """Pad/cast/shard the inputs; returns per-core in_maps."""
    x = np.asarray(x, dtype=np.float32)
    codebook = np.asarray(codebook, dtype=np.float32)

    xb = x.astype(ml_dtypes.bfloat16)
    xw = np.zeros((B, P, TP), dtype=ml_dtypes.bfloat16)
    xw[:, :, PAD : PAD + T] = xb

    # value of the bf16-rounded codebook, exactly scaled by -2
    cbb = codebook.astype(ml_dtypes.bfloat16).astype(np.float32)
    rhs = np.zeros((128, NCHUNK * K), dtype=np.float32)
    for c in range(NCHUNK):
        rhs[:CHROWS, c * K : (c + 1) * K] = -2.0 * cbb[:, CHROWS * c : CHROWS * (c + 1)].T
    rhs_bf = rhs.astype(ml_dtypes.bfloat16)

    tau = np.arange(TP, dtype=np.float32)
    cnt = np.minimum(np.minimum(tau + 1.0, float(WIN)), float(TP) - tau)
    cnt_rep = np.tile(cnt.reshape(4, TCHUNK), (BC * P, 1)).astype(np.float32)

    in_maps = []
    for i in range(NCORES):
        in_maps.append(
            {
                "xw": np.ascontiguousarray(xw[BC * i : BC * (i + 1)]),
                "cb": rhs_bf,
                "cnt": cnt_rep,
            }
        )
    return in_maps


def kernel(x, codebook):
    from concourse.bass_utils import run_bass_kernel_spmd

    nc = get_nc()
    in_maps = _host_prep(x, codebook)
    res = run_bass_kernel_spmd(nc, in_maps, core_ids=list(range(NCORES)))
    total = np.float64(0.0)
    for r in res.results:
        total += np.float64(r["out"][0, 0])
    return np.array(np.float32(total))
